# revision 2
# baseline (speedup 1.0000x reference)
"""Trainium2 Bass kernel for the AttZAM attention-weight module.

Computation (full shapes):
    trans_q[b,j,a] = sum_k w_f[j,a,k] * emb_q[b,k]        b=256, j=256, a=128, k=256
    h[b,j,a]      = tanh(trans_q + b_f[j,a])
    g[b,j]        = sum_a h[b,j,a] * w_h[a,0]
    out[b,l]      = sum_j emb_iseq[b,l,j] * g[b,j]        l=1024

Sharding: the j axis (256) is split 8 ways (32 j's per core).  Each core
computes g[b, j_slice] for ALL b, then the partial contraction
sum_{j in slice} emb_iseq[b,l,j] * g[b,j] for all (b,l).  The host sums the
8 partial outputs.  No collectives needed.

Per-core kernel layout:
  Phase A: matmul  lhsT=W_cT[k,ja] (bf16), rhs=emb_q.T[k,b] -> psum [ja_chunk=128, b=256]
           tanh(+per-partition bias) on ScalarE -> h [a=128, b=256] bf16
           matmul  lhsT=h[:, b_chunk] (M=128), rhs=w_h [a,1] (N=1) -> g column [b=128, 1]
           32 columns accumulate into psum g [b=128, j=32] per b_chunk.
  Phase B: for each j': build D = diag(g[:, j']) via tensor_scalar_mul(identity, g-col);
           matmul psum[b=128, l=512] += D.T @ E_perm[j', b_chunk, l_chunk]
           accumulating over all 32 j' in 4 held psum banks -> copy -> DMA out.
"""

import os
import sys

import numpy as np
import ml_dtypes

sys.path.insert(0, "/opt/trn_rl_repo")

import concourse.bass as bass  # noqa: E402
import concourse.mybir as mybir  # noqa: E402
import concourse.tile as tile  # noqa: E402
from concourse import bacc  # noqa: E402
from concourse.bass_utils import run_bass_kernel_spmd  # noqa: E402
from concourse.masks import make_identity  # noqa: E402

N_CORES = 8
BSZ, MAX_LEN, D, D_ATTN = 256, 1024, 256, 128
JS = D // N_CORES          # 32 j's per core
JA = JS * D_ATTN           # 4096 rows of the per-core W slice
P = 128                    # partitions
KC = D // P                # 2 k-chunks
NB = BSZ // P              # 2 b-chunks
JG = 4                     # j's per E-tile DMA (1 MiB per DMA)
LCH = 512                  # l-chunk (one fp32 psum bank)
NL = MAX_LEN // LCH        # 2 l-chunks

BF16 = mybir.dt.bfloat16
F32 = mybir.dt.float32
bf16_np = ml_dtypes.bfloat16

_CACHED_NC = None


def build_nc():
    nc = bacc.Bacc(
        "TRN2",
        target_bir_lowering=False,
        debug=False,
        num_devices=N_CORES,
    )

    w_t = nc.dram_tensor("w_t", [D, JA], BF16, kind="ExternalInput")          # [k, ja]
    q_t = nc.dram_tensor("q_t", [D, BSZ], BF16, kind="ExternalInput")         # [k, b]
    bias = nc.dram_tensor("bias", [D_ATTN, JS], F32, kind="ExternalInput")    # [a, j']
    wh = nc.dram_tensor("wh", [D_ATTN, 1], BF16, kind="ExternalInput")        # [a, 1]
    e = nc.dram_tensor("e", [NB, P, JS, MAX_LEN], BF16, kind="ExternalInput")
    out = nc.dram_tensor("out", [BSZ, MAX_LEN], F32, kind="ExternalOutput")

    with tile.TileContext(nc) as tc:
        with (
            tc.tile_pool(name="const", bufs=1) as cpool,
            tc.tile_pool(name="epool", bufs=3) as epool,
            tc.tile_pool(name="hpool", bufs=4) as hpool,
            tc.tile_pool(name="dpool", bufs=4) as dpool,
            tc.tile_pool(name="opool", bufs=2) as opool,
            tc.tile_pool(name="psA", bufs=2, space="PSUM") as psa_pool,
            tc.tile_pool(name="psG", bufs=1, space="PSUM") as psg_pool,
            tc.tile_pool(name="psB", bufs=1, space="PSUM") as psb_pool,
        ):
            # ---- constants / small inputs ----
            w_sb = []
            q_sb = []
            for kc in range(KC):
                wt = cpool.tile([P, JA], BF16, tag=f"w{kc}", name=f"w_sb{kc}")
                nc.sync.dma_start(out=wt, in_=w_t[kc * P : (kc + 1) * P, :])
                w_sb.append(wt)
                qt = cpool.tile([P, BSZ], BF16, tag=f"q{kc}", name=f"q_sb{kc}")
                nc.sync.dma_start(out=qt, in_=q_t[kc * P : (kc + 1) * P, :])
                q_sb.append(qt)
            bias_sb = cpool.tile([D_ATTN, JS], F32, tag="bias", name="bias_sb")
            nc.sync.dma_start(out=bias_sb, in_=bias[:, :])
            wh_sb = cpool.tile([D_ATTN, 1], BF16, tag="wh", name="wh_sb")
            nc.sync.dma_start(out=wh_sb, in_=wh[:, :])
            ident = cpool.tile([P, P], BF16, tag="ident", name="ident")
            make_identity(nc, ident)

            g_sb = [
                cpool.tile([P, JS], F32, tag=f"g{bc}", name=f"g_sb{bc}")
                for bc in range(NB)
            ]

            # ---- Phase A: g[b, j'] for the core's j-slice ----
            g_ps = [
                psg_pool.tile([P, JS], F32, tag=f"gps{bc}", name=f"g_ps{bc}")
                for bc in range(NB)
            ]
            for jp in range(JS):
                ps = psa_pool.tile([P, BSZ], F32, tag="psA", name="psA")
                for kc in range(KC):
                    nc.tensor.matmul(
                        ps,
                        w_sb[kc][:, jp * P : (jp + 1) * P],
                        q_sb[kc],
                        start=(kc == 0),
                        stop=(kc == KC - 1),
                    )
                h = hpool.tile([P, BSZ], BF16, tag="h", name="h")
                nc.scalar.activation(
                    h,
                    ps,
                    mybir.ActivationFunctionType.Tanh,
                    bias=bias_sb[:, jp : jp + 1],
                )
                for bc in range(NB):
                    nc.tensor.matmul(
                        g_ps[bc][:, jp : jp + 1],
                        h[:, bc * P : (bc + 1) * P],
                        wh_sb,
                        start=True,
                        stop=True,
                    )
            for bc in range(NB):
                nc.vector.tensor_copy(g_sb[bc], g_ps[bc])

            # ---- Phase B: partial out[b, l] accumulated over j' ----
            ps_out = [
                [
                    psb_pool.tile([P, LCH], F32, tag=f"psB{bc}_{lc}", name=f"psB{bc}_{lc}")
                    for lc in range(NL)
                ]
                for bc in range(NB)
            ]
            for jg in range(JS // JG):
                etiles = []
                for bc in range(NB):
                    et = epool.tile([P, JG, MAX_LEN], BF16, tag=f"e{bc}", name=f"et{bc}")
                    nc.sync.dma_start(
                        out=et, in_=e[bc, :, jg * JG : (jg + 1) * JG, :]
                    )
                    etiles.append(et)
                for jj in range(JG):
                    jp = jg * JG + jj
                    for bc in range(NB):
                        d = dpool.tile([P, P], BF16, tag=f"d{bc}", name=f"d{bc}")
                        nc.vector.tensor_scalar_mul(d, ident, g_sb[bc][:, jp : jp + 1])
                        for lc in range(NL):
                            nc.tensor.matmul(
                                ps_out[bc][lc],
                                d,
                                etiles[bc][:, jj, lc * LCH : (lc + 1) * LCH],
                                start=(jp == 0),
                                stop=(jp == JS - 1),
                            )
            for bc in range(NB):
                o = opool.tile([P, MAX_LEN], F32, tag="o", name="o")
                for lc in range(NL):
                    nc.vector.tensor_copy(o[:, lc * LCH : (lc + 1) * LCH], ps_out[bc][lc])
                nc.sync.dma_start(out=out[bc * P : (bc + 1) * P, :], in_=o)

    nc.compile()
    return nc


def get_nc():
    global _CACHED_NC
    if _CACHED_NC is None:
        _CACHED_NC = build_nc()
    return _CACHED_NC


def make_in_maps(emb_q, emb_iseq, w_f, b_f, w_h):
    """Host-side shard + layout + bf16 cast.  Returns list of per-core dicts."""
    q_t = np.ascontiguousarray(emb_q.astype(np.float32).T).astype(bf16_np)  # [k, b]
    wh = np.ascontiguousarray(w_h.astype(np.float32)).astype(bf16_np)       # [a, 1]
    in_maps = []
    for c in range(N_CORES):
        js, je = c * JS, (c + 1) * JS
        w_slice = w_f[js:je].reshape(JA, D)                       # [ja, k]
        w_t = np.ascontiguousarray(w_slice.T).astype(bf16_np)     # [k, ja]
        bias = np.ascontiguousarray(b_f[js:je].T.astype(np.float32))  # [a, j']
        e_perm = emb_iseq[:, :, js:je].transpose(0, 2, 1)         # [b, j', l]
        e_arr = np.ascontiguousarray(e_perm).astype(bf16_np).reshape(
            NB, P, JS, MAX_LEN
        )
        in_maps.append(
            {"w_t": w_t, "q_t": q_t, "bias": bias, "wh": wh, "e": e_arr}
        )
    return in_maps


def run(in_maps, trace=False, **kwargs):
    nc = get_nc()
    return run_bass_kernel_spmd(
        nc, in_maps, core_ids=list(range(N_CORES)), trace=trace, **kwargs
    )


def kernel(emb_q, emb_iseq, w_f, b_f, w_h):
    in_maps = make_in_maps(emb_q, emb_iseq, w_f, b_f, w_h)
    res = run(in_maps, trace=False)
    partial = np.zeros((BSZ, MAX_LEN), dtype=np.float32)
    for r in res.results:
        partial += r["out"]
    return partial


# revision 3
# speedup vs baseline: 1.1293x; 1.1293x over previous
"""Trainium2 Bass kernel for the AttZAM attention-weight module.

Computation (full shapes):
    trans_q[b,j,a] = sum_k w_f[j,a,k] * emb_q[b,k]        b=256, j=256, a=128, k=256
    h[b,j,a]      = tanh(trans_q + b_f[j,a])
    g[b,j]        = sum_a h[b,j,a] * w_h[a,0]
    out[b,l]      = sum_j emb_iseq[b,l,j] * g[b,j]        l=1024

Sharding: the j axis (256) is split 8 ways (32 j's per core).  Each core
computes g[b, j_slice] for ALL b, then the partial contraction
sum_{j in slice} emb_iseq[b,l,j] * g[b,j] for all (b,l).  The host sums the
8 partial outputs.  No collectives needed.

Per-core kernel layout:
  Phase A: matmul  lhsT=W_cT[k,ja] (bf16), rhs=emb_q.T[k,b] -> psum [ja_chunk=128, b=256]
           tanh(+per-partition bias) on ScalarE -> h [a=128, b=256] bf16
           matmul  lhsT=h[:, b_chunk] (M=128), rhs=w_h [a,1] (N=1) -> g column [b=128, 1]
           32 columns accumulate into psum g [b=128, j=32] per b_chunk.
  Phase B: for each j': build D = diag(g[:, j']) via tensor_scalar_mul(identity, g-col);
           matmul psum[b=128, l=512] += D.T @ E_perm[j', b_chunk, l_chunk]
           accumulating over all 32 j' in 4 held psum banks -> copy -> DMA out.
"""

import os
import sys

import numpy as np
import ml_dtypes

sys.path.insert(0, "/opt/trn_rl_repo")

import concourse.bass as bass  # noqa: E402
import concourse.mybir as mybir  # noqa: E402
import concourse.tile as tile  # noqa: E402
from concourse import bacc  # noqa: E402
from concourse.bass_utils import run_bass_kernel_spmd  # noqa: E402
from concourse.masks import make_identity  # noqa: E402

N_CORES = 8
BSZ, MAX_LEN, D, D_ATTN = 256, 1024, 256, 128
JS = D // N_CORES          # 32 j's per core
JA = JS * D_ATTN           # 4096 rows of the per-core W slice
P = 128                    # partitions
KC = D // P                # 2 k-chunks
NB = BSZ // P              # 2 b-chunks
JG = 4                     # j's per E-tile DMA (1 MiB per DMA)
LCH = 512                  # l-chunk (one fp32 psum bank)
NL = MAX_LEN // LCH        # 2 l-chunks

BF16 = mybir.dt.bfloat16
F32 = mybir.dt.float32
bf16_np = ml_dtypes.bfloat16

_CACHED_NC = None


def build_nc():
    nc = bacc.Bacc(
        "TRN2",
        target_bir_lowering=False,
        debug=False,
        num_devices=N_CORES,
    )

    w_t = nc.dram_tensor("w_t", [D, JA], BF16, kind="ExternalInput")          # [k, ja]
    q_t = nc.dram_tensor("q_t", [D, BSZ], BF16, kind="ExternalInput")         # [k, b]
    bias = nc.dram_tensor("bias", [D_ATTN, JS], F32, kind="ExternalInput")    # [a, j']
    wh = nc.dram_tensor("wh", [D_ATTN, 1], BF16, kind="ExternalInput")        # [a, 1]
    e = nc.dram_tensor("e", [NB, P, JS, MAX_LEN], BF16, kind="ExternalInput")
    out = nc.dram_tensor("out", [BSZ, MAX_LEN], F32, kind="ExternalOutput")

    with tile.TileContext(nc) as tc:
        with (
            tc.tile_pool(name="const", bufs=1) as cpool,
            tc.tile_pool(name="wpool", bufs=3) as wpool,
            tc.tile_pool(name="epool", bufs=4) as epool,
            tc.tile_pool(name="hpool", bufs=4) as hpool,
            tc.tile_pool(name="dpool", bufs=4) as dpool,
            tc.tile_pool(name="opool", bufs=2) as opool,
            tc.tile_pool(name="psA", bufs=2, space="PSUM") as psa_pool,
            tc.tile_pool(name="psG", bufs=1, space="PSUM") as psg_pool,
            tc.tile_pool(name="psB", bufs=1, space="PSUM") as psb_pool,
        ):
            # ---- constants / small inputs ----
            q_sb = []
            for kc in range(KC):
                qt = cpool.tile([P, BSZ], BF16, tag=f"q{kc}", name=f"q_sb{kc}")
                nc.sync.dma_start(out=qt, in_=q_t[kc * P : (kc + 1) * P, :])
                q_sb.append(qt)
            bias_sb = cpool.tile([D_ATTN, JS], F32, tag="bias", name="bias_sb")
            nc.sync.dma_start(out=bias_sb, in_=bias[:, :])
            wh_sb = cpool.tile([D_ATTN, 1], BF16, tag="wh", name="wh_sb")
            nc.sync.dma_start(out=wh_sb, in_=wh[:, :])
            ident = cpool.tile([P, P], BF16, tag="ident", name="ident")
            make_identity(nc, ident)

            g_sb = [
                cpool.tile([P, JS], F32, tag=f"g{bc}", name=f"g_sb{bc}")
                for bc in range(NB)
            ]
            g_ps = [
                psg_pool.tile([P, JS], F32, tag=f"gps{bc}", name=f"g_ps{bc}")
                for bc in range(NB)
            ]
            ps_out = [
                [
                    psb_pool.tile([P, LCH], F32, tag=f"psB{bc}_{lc}", name=f"psB{bc}_{lc}")
                    for lc in range(NL)
                ]
                for bc in range(NB)
            ]

            # ---- interleaved phase A / phase B, one j-group at a time ----
            JW = JG * D_ATTN  # ja columns per group
            for jg in range(JS // JG):
                # group's slice of W (so phase A starts without the full-W DMA)
                w_g = []
                for kc in range(KC):
                    wt = wpool.tile([P, JW], BF16, tag=f"w{kc}", name=f"w_g{kc}")
                    nc.sync.dma_start(
                        out=wt,
                        in_=w_t[kc * P : (kc + 1) * P, jg * JW : (jg + 1) * JW],
                    )
                    w_g.append(wt)
                etiles = []
                for bc in range(NB):
                    et = epool.tile([P, JG, MAX_LEN], BF16, tag=f"e{bc}", name=f"et{bc}")
                    nc.sync.dma_start(
                        out=et, in_=e[bc, :, jg * JG : (jg + 1) * JG, :]
                    )
                    etiles.append(et)

                # phase A for this group's JG j's
                for jj in range(JG):
                    jp = jg * JG + jj
                    ps = psa_pool.tile([P, BSZ], F32, tag="psA", name="psA")
                    for kc in range(KC):
                        nc.tensor.matmul(
                            ps,
                            w_g[kc][:, jj * P : (jj + 1) * P],
                            q_sb[kc],
                            start=(kc == 0),
                            stop=(kc == KC - 1),
                        )
                    h = hpool.tile([P, BSZ], BF16, tag="h", name="h")
                    nc.scalar.activation(
                        h,
                        ps,
                        mybir.ActivationFunctionType.Tanh,
                        bias=bias_sb[:, jp : jp + 1],
                    )
                    for bc in range(NB):
                        nc.tensor.matmul(
                            g_ps[bc][:, jp : jp + 1],
                            h[:, bc * P : (bc + 1) * P],
                            wh_sb,
                            start=True,
                            stop=True,
                        )
                # g for this group -> SBUF
                for bc in range(NB):
                    nc.vector.tensor_copy(
                        g_sb[bc][:, jg * JG : (jg + 1) * JG],
                        g_ps[bc][:, jg * JG : (jg + 1) * JG],
                    )
                # phase B for this group
                for jj in range(JG):
                    jp = jg * JG + jj
                    for bc in range(NB):
                        d = dpool.tile([P, P], BF16, tag=f"d{bc}", name=f"d{bc}")
                        nc.vector.tensor_scalar_mul(d, ident, g_sb[bc][:, jp : jp + 1])
                        for lc in range(NL):
                            nc.tensor.matmul(
                                ps_out[bc][lc],
                                d,
                                etiles[bc][:, jj, lc * LCH : (lc + 1) * LCH],
                                start=(jp == 0),
                                stop=(jp == JS - 1),
                            )

            # ---- tail: psum -> sbuf -> dram, split per (bc, lc) ----
            for bc in range(NB):
                for lc in range(NL):
                    o = opool.tile([P, LCH], F32, tag=f"o{bc}_{lc}", name=f"o{bc}_{lc}")
                    if bc == 0:
                        nc.vector.tensor_copy(o, ps_out[bc][lc])
                    else:
                        nc.scalar.copy(o, ps_out[bc][lc])
                    nc.sync.dma_start(
                        out=out[bc * P : (bc + 1) * P, lc * LCH : (lc + 1) * LCH],
                        in_=o,
                    )

    nc.compile()
    return nc


def get_nc():
    global _CACHED_NC
    if _CACHED_NC is None:
        _CACHED_NC = build_nc()
    return _CACHED_NC


def make_in_maps(emb_q, emb_iseq, w_f, b_f, w_h):
    """Host-side shard + layout + bf16 cast.  Returns list of per-core dicts."""
    q_t = np.ascontiguousarray(emb_q.astype(np.float32).T).astype(bf16_np)  # [k, b]
    wh = np.ascontiguousarray(w_h.astype(np.float32)).astype(bf16_np)       # [a, 1]
    in_maps = []
    for c in range(N_CORES):
        js, je = c * JS, (c + 1) * JS
        w_slice = w_f[js:je].reshape(JA, D)                       # [ja, k]
        w_t = np.ascontiguousarray(w_slice.T).astype(bf16_np)     # [k, ja]
        bias = np.ascontiguousarray(b_f[js:je].T.astype(np.float32))  # [a, j']
        e_perm = emb_iseq[:, :, js:je].transpose(0, 2, 1)         # [b, j', l]
        e_arr = np.ascontiguousarray(e_perm).astype(bf16_np).reshape(
            NB, P, JS, MAX_LEN
        )
        in_maps.append(
            {"w_t": w_t, "q_t": q_t, "bias": bias, "wh": wh, "e": e_arr}
        )
    return in_maps


def run(in_maps, trace=False, **kwargs):
    nc = get_nc()
    return run_bass_kernel_spmd(
        nc, in_maps, core_ids=list(range(N_CORES)), trace=trace, **kwargs
    )


def kernel(emb_q, emb_iseq, w_f, b_f, w_h):
    in_maps = make_in_maps(emb_q, emb_iseq, w_f, b_f, w_h)
    res = run(in_maps, trace=False)
    partial = np.zeros((BSZ, MAX_LEN), dtype=np.float32)
    for r in res.results:
        partial += r["out"]
    return partial
